# revision 45
# baseline (speedup 1.0000x reference)
"""Trainium2 Bass kernel for nn_CP_Based (CP-decomposition feature-product layer).

Math: out[b,u] = sum_r prod_f ( x0[b,f]*K[0,r,f,u] + x1[b,f]*K[1,r,f,u] )
  with x0 = 1/sqrt(1+X^2), x1 = X/sqrt(1+X^2).
Factor the normalization out of the f-product:
  out[b,u] = S[b] * sum_r prod_f ( K0[f,ru] + X[b,f]*K1[f,ru] ),
  S[b] = 1/sqrt(prod_f (1+X[b,f]^2)).
S and the final sum over CP rank are cheap per-row postprocessing and are
applied on the host; the device computes the unnormalized rank products.

The 32-feature product is decomposed into 8 groups of 4 features. Each group's
product is a linear map from the 16 multilinear monomials of its 4 features:
  G_g[b,ru] = sum_m Q_g[b,m] * C_g[m,ru]        (K=32 f32r matmul on TensorE)
with C_g packed on the host (zero rows pad each group pair to 32 so every
matmul slice is 32-partition aligned). Monomials Q are built in-place per
512-row macro tile (f32r), transposed via TensorE into a PSUM tile so the
monomial index lands on the contraction axis, copied once to SBUF, then 8
matmuls produce the G_g into two 2-bank PSUM pair tiles; the even pairs are
evacuated to bf16 SBUF (ScalarE) and a 3-level multiply tree on DVE/Pool
forms prod_g G_g, DMA'd out as bf16.

Sharding: pure data-parallel over batch: 131072 rows -> 8 cores x 16384.
"""

import sys

import numpy as np

sys.path.insert(0, "/opt/trn_rl_repo")

import concourse.bacc as bacc  # noqa: E402
import concourse.mybir as mybir  # noqa: E402
from concourse.bass_utils import run_bass_kernel_spmd  # noqa: E402
from concourse.tile import TileContext  # noqa: E402

F32 = mybir.dt.float32
F32R = mybir.dt.float32r
BF16 = mybir.dt.bfloat16
AF = mybir.ActivationFunctionType
OP = mybir.AluOpType
AX = mybir.AxisListType

B_FULL = 131072
N_CORES = 8
B_CORE = B_FULL // N_CORES  # 16384
F = 32
R, U = 10, 8
RU = R * U  # 80
NG = 8  # feature groups of 4
TILE_B = 128
CHUNK = 4  # b-subtiles per macro tile
MACRO_B = TILE_B * CHUNK  # 512
N_MACRO = B_CORE // MACRO_B  # 32
N_M2 = N_MACRO // 2  # 16 two-macro groups (DMA batching)
CG = CHUNK * NG  # 32 (chunk, group) pairs
QBUFS = 4


def build_nc():
    nc = bacc.Bacc()
    # host pre-arranges X as [m2, partition, 2*chunk, feature] so each
    # 2-macro load is one contiguous 128 KB DMA
    X = nc.dram_tensor(
        "X", [N_M2, TILE_B, 2 * CHUNK, F], F32, kind="ExternalInput"
    )
    # C | identity | ones packed as one constant block: single DMA at startup
    CONSTW = 4 * RU + 128 + CG
    consts = nc.dram_tensor("consts", [128, CONSTW], F32R, kind="ExternalInput")
    out = nc.dram_tensor(
        "out", [N_MACRO, RU, 4, MACRO_B], BF16, kind="ExternalOutput"
    )

    with TileContext(nc) as tc:
        with (
            tc.tile_pool(name="const", bufs=1) as cpool,
            tc.tile_pool(name="xin", bufs=4) as xpool,
            tc.tile_pool(name="work", bufs=QBUFS) as wpool,
            tc.tile_pool(name="qts", bufs=4) as qpool,
            tc.tile_pool(name="ps_t", bufs=1, space="PSUM") as tps,
            tc.tile_pool(name="ps_g", bufs=1, space="PSUM") as gps,
        ):
            # first X block before the consts so its DMA-sem lands earliest
            x2_0 = xpool.tile([TILE_B, 2 * CHUNK, F], F32, tag="x", name="x2")
            nc.sync.dma_start(out=x2_0[:], in_=X[0])
            cst = cpool.tile([128, 4 * RU + 128 + CG], F32R, tag="cst")
            nc.sync.dma_start(out=cst[:], in_=consts[:, :])
            c_sb = cst[:, 0 : 4 * RU]
            id_sb = cst[:, 4 * RU : 4 * RU + 128]
            on_sb = cst[:, 4 * RU + 128 :]

            # warm up the PE p-state during the const-DMA window so the
            # first real transposes/matmuls run at full clock
            for _ in range(16):
                warm = gps.tile([128, MACRO_B], F32R, tag="gO1", name="warm", bufs=3)
                nc.tensor.transpose(warm[:, 0:128].bitcast(F32R), id_sb, id_sb)

            # the constant monomial slot q[:, :, 0, 0] never changes after the
            # fold moved to the host: write it once per physical buffer
            for _ in range(QBUFS):
                qinit = wpool.tile([TILE_B, CG, 4, 4], F32R, tag="q", name="qinit")
                nc.vector.tensor_copy(
                    qinit[:, :, 0:1, 0:1], on_sb.unsqueeze(2).unsqueeze(3)
                )

            for mi in range(N_MACRO):
                m2, hh = mi // 2, mi % 2
                if mi == 0:
                    x2 = x2_0
                elif hh == 0:
                    x2 = xpool.tile([TILE_B, 2 * CHUNK, F], F32, tag="x", name="x2")
                    nc.sync.dma_start(out=x2[:], in_=X[m2])
                xm = x2[:, 4 * hh : 4 * hh + 4, :]  # [128, 4, 32]
                hp = tc.high_priority(offset=25)
                hp.__enter__()

                # --- monomials built in-place in q[128, cg, 4, 4] ---
                # q[b, cg, i, j] = pab_i(b,cg) * pcd_j(b,cg) on raw X
                q = wpool.tile([TILE_B, CG, 4, 4], F32R, tag="q")
                xg = xm.rearrange("p c (g j) -> p (c g) j", j=4)
                # pab column (j=0): [1, Xa, Xb, XaXb]
                nc.gpsimd.tensor_copy(q[:, :, 1:3, 0:1], xg[:, :, 0:2].unsqueeze(3))
                nc.gpsimd.tensor_mul(
                    q[:, :, 3:4, 0:1],
                    xg[:, :, 0:1].unsqueeze(3),
                    xg[:, :, 1:2].unsqueeze(3),
                )
                # pcd row (i=0): [1, Xc, Xd, XcXd]
                nc.gpsimd.tensor_copy(q[:, :, 0:1, 1:3], xg[:, :, 2:4].unsqueeze(2))
                nc.gpsimd.tensor_mul(
                    q[:, :, 0:1, 3:4],
                    xg[:, :, 2:3].unsqueeze(2),
                    xg[:, :, 3:4].unsqueeze(2),
                )
                # outer product fills i>=1, j>=1
                nc.gpsimd.tensor_tensor(
                    q[:, :, 1:4, 1:4],
                    q[:, :, 1:4, 0:1].broadcast_to([TILE_B, CG, 3, 3]),
                    q[:, :, 0:1, 1:4].broadcast_to([TILE_B, CG, 3, 3]),
                    OP.mult,
                )

                # --- transpose Q (one [128,128] per chunk) -> PSUM ---
                qf = q[:].rearrange("p cg i j -> p (cg i j)")  # [128, 512]
                ps_a = tps.tile([128, MACRO_B], F32R, tag="ps_a")
                for c in range(CHUNK):
                    cw = slice(c * TILE_B, (c + 1) * TILE_B)
                    nc.tensor.transpose(
                        ps_a[:, cw], qf[:, c * 128 : (c + 1) * 128], id_sb
                    )

                # --- copy QT to SBUF (one wide op) ---
                qt = qpool.tile([128, MACRO_B], F32R, tag="qt")
                nc.scalar.copy(qt[:], ps_a[:])
                hp.__exit__(None, None, None)

                # --- 8 group matmuls (K=64): evens into a 2-bank pair tile
                # (batched Act evacuation), odds into single-bank tiles; all
                # four pair products on DVE (GPSIMD cannot touch PSUM).
                # t is DMA'd out; the host multiplies the four t's and sums
                # over rank.
                t_sb = qpool.tile([RU, 4, MACRO_B], BF16, tag="t_sb")
                for half in range(2):
                    pE = gps.tile(
                        [RU, 2, MACRO_B], F32, tag=f"pE{half}", name=f"pE{half}",
                        bufs=1,
                    )
                    eE = qpool.tile(
                        [RU, 2, MACRO_B], BF16, tag=f"eE{half}", name=f"eE{half}"
                    )
                    gOs = []
                    for par in range(2):
                        g4 = 4 * half + 2 * par
                        base = 64 * (g4 // 4)
                        nc.tensor.matmul(
                            pE[:, par, :],
                            c_sb[base : base + 64, RU * (g4 % 4) : RU * (g4 % 4 + 1)],
                            qt[base : base + 64, :],
                            start=True,
                            stop=True,
                        )
                        gO1 = gps.tile(
                            [RU, MACRO_B], F32, tag="gO1", name="gO1", bufs=3
                        )
                        g = g4 + 1
                        nc.tensor.matmul(
                            gO1[:],
                            c_sb[base : base + 64, RU * (g % 4) : RU * (g % 4 + 1)],
                            qt[base : base + 64, :],
                            start=True,
                            stop=True,
                        )
                        gOs.append(gO1)
                    nc.scalar.copy(eE[:], pE[:])
                    for par in range(2):
                        nc.vector.tensor_tensor(
                            t_sb[:, 2 * half + par, :],
                            eE[:, par, :],
                            gOs[par][:],
                            OP.mult,
                        )
                nc.sync.dma_start(out=out[mi], in_=t_sb[:])
    nc.finalize()
    return nc


def _pack_weights(kernel: np.ndarray):
    K = kernel.astype(np.float32)  # [2, R, F, U]
    C = np.zeros((128, 4 * RU), np.float32)
    bits = [(0, 0), (1, 0), (0, 1), (1, 1)]
    for g in range(NG):
        r0 = 16 * g
        c0 = RU * (g % 4)
        fs = [4 * g, 4 * g + 1, 4 * g + 2, 4 * g + 3]
        for i, (ba, bb) in enumerate(bits):
            for j, (bc, bd) in enumerate(bits):
                coef = (
                    K[ba, :, fs[0], :]
                    * K[bb, :, fs[1], :]
                    * K[bc, :, fs[2], :]
                    * K[bd, :, fs[3], :]
                )  # [R, U]
                C[r0 + i * 4 + j, c0 : c0 + RU] = coef.reshape(RU)
    ident = np.eye(128, dtype=np.float32)
    ones = np.ones((128, CG), np.float32)
    consts = np.concatenate([C, ident, ones], axis=1)
    return consts


_NC_CACHE = {}


def kernel(X: np.ndarray, kernel: np.ndarray) -> np.ndarray:
    if "nc" not in _NC_CACHE:
        _NC_CACHE["nc"] = build_nc()
    nc = _NC_CACHE["nc"]
    consts = _pack_weights(kernel)
    X = np.ascontiguousarray(X, dtype=np.float32)
    # [core, m2, half, chunk, partition, F] -> [core, m2, partition, half*chunk, F]
    Xd = (
        X.reshape(N_CORES, N_M2, 2 * CHUNK, TILE_B, F)
        .transpose(0, 1, 3, 2, 4)
        .copy()
    )
    in_maps = []
    for c in range(N_CORES):
        in_maps.append({"X": Xd[c], "consts": consts})
    res = run_bass_kernel_spmd(nc, in_maps, core_ids=list(range(N_CORES)))

    # host epilogue: product of the four pair-products, sum over CP rank,
    # then apply the normalization S[b] = 1/sqrt(prod_f(1+X^2))
    outs = []
    for c in range(N_CORES):
        o = res.results[c]["out"]  # [N_MACRO, RU, 4, MACRO_B] bf16
        o = np.asarray(o, dtype=np.float32)
        o = o[:, :, 0] * o[:, :, 1] * o[:, :, 2] * o[:, :, 3]
        o = o.reshape(N_MACRO, R, U, MACRO_B).sum(axis=1)  # [N_MACRO, U, 512]
        outs.append(o.transpose(0, 2, 1).reshape(B_CORE, U))
    raw = np.concatenate(outs, axis=0)  # [B, U], rows in device order
    S = 1.0 / np.sqrt(np.prod(1.0 + X.astype(np.float64) ** 2, axis=1))
    return (raw * S[:, None].astype(np.float32)).astype(np.float32)


if __name__ == "__main__":
    rng = np.random.default_rng(0)
    X = rng.standard_normal((B_FULL, F), dtype=np.float32)
    K = (rng.standard_normal((2, R, F, U)) * 0.24).astype(np.float32)
    y = kernel(X, K)
    print(y.shape, y.dtype, np.abs(y).max())


# revision 46
# speedup vs baseline: 1.0045x; 1.0045x over previous
"""Trainium2 Bass kernel for nn_CP_Based (CP-decomposition feature-product layer).

Math: out[b,u] = sum_r prod_f ( x0[b,f]*K[0,r,f,u] + x1[b,f]*K[1,r,f,u] )
  with x0 = 1/sqrt(1+X^2), x1 = X/sqrt(1+X^2).
Factor the normalization out of the f-product:
  out[b,u] = S[b] * sum_r prod_f ( K0[f,ru] + X[b,f]*K1[f,ru] ),
  S[b] = 1/sqrt(prod_f (1+X[b,f]^2)).
S and the final sum over CP rank are cheap per-row postprocessing and are
applied on the host; the device computes the unnormalized rank products.

The 32-feature product is decomposed into 8 groups of 4 features. Each group's
product is a linear map from the 16 multilinear monomials of its 4 features:
  G_g[b,ru] = sum_m Q_g[b,m] * C_g[m,ru]        (K=32 f32r matmul on TensorE)
with C_g packed on the host (zero rows pad each group pair to 32 so every
matmul slice is 32-partition aligned). Monomials Q are built in-place per
512-row macro tile (f32r), transposed via TensorE into a PSUM tile so the
monomial index lands on the contraction axis, copied once to SBUF, then 8
matmuls produce the G_g into two 2-bank PSUM pair tiles; the even pairs are
evacuated to bf16 SBUF (ScalarE) and a 3-level multiply tree on DVE/Pool
forms prod_g G_g, DMA'd out as bf16.

Sharding: pure data-parallel over batch: 131072 rows -> 8 cores x 16384.
"""

import sys

import numpy as np

sys.path.insert(0, "/opt/trn_rl_repo")

import concourse.bacc as bacc  # noqa: E402
import concourse.mybir as mybir  # noqa: E402
from concourse.bass_utils import run_bass_kernel_spmd  # noqa: E402
from concourse.tile import TileContext  # noqa: E402

F32 = mybir.dt.float32
F32R = mybir.dt.float32r
BF16 = mybir.dt.bfloat16
AF = mybir.ActivationFunctionType
OP = mybir.AluOpType
AX = mybir.AxisListType

B_FULL = 131072
N_CORES = 8
B_CORE = B_FULL // N_CORES  # 16384
F = 32
R, U = 10, 8
RU = R * U  # 80
NG = 8  # feature groups of 4
TILE_B = 128
CHUNK = 4  # b-subtiles per macro tile
MACRO_B = TILE_B * CHUNK  # 512
N_MACRO = B_CORE // MACRO_B  # 32
N_M2 = N_MACRO // 2  # 16 two-macro groups (DMA batching)
CG = CHUNK * NG  # 32 (chunk, group) pairs
QBUFS = 4


def build_nc():
    nc = bacc.Bacc()
    # host pre-arranges X as [m2, partition, 2*chunk, feature] so each
    # 2-macro load is one contiguous 128 KB DMA
    X = nc.dram_tensor(
        "X", [N_M2, TILE_B, 2 * CHUNK, F], F32, kind="ExternalInput"
    )
    # C | identity | ones packed as one constant block: single DMA at startup
    CONSTW = 4 * RU + 128 + CG
    consts = nc.dram_tensor("consts", [128, CONSTW], F32R, kind="ExternalInput")
    out = nc.dram_tensor(
        "out", [N_MACRO, RU, 4, MACRO_B], BF16, kind="ExternalOutput"
    )

    with TileContext(nc) as tc:
        with (
            tc.tile_pool(name="const", bufs=1) as cpool,
            tc.tile_pool(name="xin", bufs=4) as xpool,
            tc.tile_pool(name="work", bufs=QBUFS) as wpool,
            tc.tile_pool(name="qts", bufs=4) as qpool,
            tc.tile_pool(name="ps_t", bufs=1, space="PSUM") as tps,
            tc.tile_pool(name="ps_g", bufs=1, space="PSUM") as gps,
        ):
            cst = cpool.tile([128, 4 * RU + 128 + CG], F32R, tag="cst")
            nc.sync.dma_start(out=cst[:], in_=consts[:, :])
            c_sb = cst[:, 0 : 4 * RU]
            id_sb = cst[:, 4 * RU : 4 * RU + 128]
            on_sb = cst[:, 4 * RU + 128 :]

            # warm up the PE p-state during the const-DMA window so the
            # first real transposes/matmuls run at full clock
            for _ in range(16):
                warm = gps.tile([128, MACRO_B], F32R, tag="gO1", name="warm", bufs=3)
                nc.tensor.transpose(warm[:, 0:128].bitcast(F32R), id_sb, id_sb)

            # the constant monomial slot q[:, :, 0, 0] never changes after the
            # fold moved to the host: write it once per physical buffer
            for _ in range(QBUFS):
                qinit = wpool.tile([TILE_B, CG, 4, 4], F32R, tag="q", name="qinit")
                nc.vector.tensor_copy(
                    qinit[:, :, 0:1, 0:1], on_sb.unsqueeze(2).unsqueeze(3)
                )

            for mi in range(N_MACRO):
                m2, hh = mi // 2, mi % 2
                if hh == 0:
                    x2 = xpool.tile([TILE_B, 2 * CHUNK, F], F32, tag="x")
                    nc.sync.dma_start(out=x2[:], in_=X[m2])
                xm = x2[:, 4 * hh : 4 * hh + 4, :]  # [128, 4, 32]
                hp = tc.high_priority(offset=25)
                hp.__enter__()

                # --- monomials built in-place in q[128, cg, 4, 4] ---
                # q[b, cg, i, j] = pab_i(b,cg) * pcd_j(b,cg) on raw X
                q = wpool.tile([TILE_B, CG, 4, 4], F32R, tag="q")
                xg = xm.rearrange("p c (g j) -> p (c g) j", j=4)
                # pab column (j=0): [1, Xa, Xb, XaXb]
                nc.gpsimd.tensor_copy(q[:, :, 1:3, 0:1], xg[:, :, 0:2].unsqueeze(3))
                nc.gpsimd.tensor_mul(
                    q[:, :, 3:4, 0:1],
                    xg[:, :, 0:1].unsqueeze(3),
                    xg[:, :, 1:2].unsqueeze(3),
                )
                # pcd row (i=0): [1, Xc, Xd, XcXd]
                nc.gpsimd.tensor_copy(q[:, :, 0:1, 1:3], xg[:, :, 2:4].unsqueeze(2))
                nc.gpsimd.tensor_mul(
                    q[:, :, 0:1, 3:4],
                    xg[:, :, 2:3].unsqueeze(2),
                    xg[:, :, 3:4].unsqueeze(2),
                )
                # outer product fills i>=1, j>=1
                nc.gpsimd.tensor_tensor(
                    q[:, :, 1:4, 1:4],
                    q[:, :, 1:4, 0:1].broadcast_to([TILE_B, CG, 3, 3]),
                    q[:, :, 0:1, 1:4].broadcast_to([TILE_B, CG, 3, 3]),
                    OP.mult,
                )

                # --- transpose Q (one [128,128] per chunk) -> PSUM ---
                qf = q[:].rearrange("p cg i j -> p (cg i j)")  # [128, 512]
                ps_a = tps.tile([128, MACRO_B], F32R, tag="ps_a")
                for c in range(CHUNK):
                    cw = slice(c * TILE_B, (c + 1) * TILE_B)
                    nc.tensor.transpose(
                        ps_a[:, cw], qf[:, c * 128 : (c + 1) * 128], id_sb
                    )

                # --- copy QT to SBUF (one wide op) ---
                qt = qpool.tile([128, MACRO_B], F32R, tag="qt")
                nc.scalar.copy(qt[:], ps_a[:])
                hp.__exit__(None, None, None)

                # --- 8 group matmuls (K=64): evens into a 2-bank pair tile
                # (batched Act evacuation), odds into single-bank tiles; all
                # four pair products on DVE (GPSIMD cannot touch PSUM).
                # t is DMA'd out; the host multiplies the four t's and sums
                # over rank.
                t_sb = qpool.tile([RU, 4, MACRO_B], BF16, tag="t_sb")
                for half in range(2):
                    pE = gps.tile(
                        [RU, 2, MACRO_B], F32, tag=f"pE{half}", name=f"pE{half}",
                        bufs=1,
                    )
                    eE = qpool.tile(
                        [RU, 2, MACRO_B], BF16, tag=f"eE{half}", name=f"eE{half}"
                    )
                    gOs = []
                    for par in range(2):
                        g4 = 4 * half + 2 * par
                        base = 64 * (g4 // 4)
                        nc.tensor.matmul(
                            pE[:, par, :],
                            c_sb[base : base + 64, RU * (g4 % 4) : RU * (g4 % 4 + 1)],
                            qt[base : base + 64, :],
                            start=True,
                            stop=True,
                        )
                        gO1 = gps.tile(
                            [RU, MACRO_B], F32, tag="gO1", name="gO1", bufs=3
                        )
                        g = g4 + 1
                        nc.tensor.matmul(
                            gO1[:],
                            c_sb[base : base + 64, RU * (g % 4) : RU * (g % 4 + 1)],
                            qt[base : base + 64, :],
                            start=True,
                            stop=True,
                        )
                        gOs.append(gO1)
                    nc.scalar.copy(eE[:], pE[:])
                    for par in range(2):
                        nc.vector.tensor_tensor(
                            t_sb[:, 2 * half + par, :],
                            eE[:, par, :],
                            gOs[par][:],
                            OP.mult,
                        )
                nc.sync.dma_start(out=out[mi], in_=t_sb[:])
    nc.finalize()
    return nc


def _pack_weights(kernel: np.ndarray):
    K = kernel.astype(np.float32)  # [2, R, F, U]
    C = np.zeros((128, 4 * RU), np.float32)
    bits = [(0, 0), (1, 0), (0, 1), (1, 1)]
    for g in range(NG):
        r0 = 16 * g
        c0 = RU * (g % 4)
        fs = [4 * g, 4 * g + 1, 4 * g + 2, 4 * g + 3]
        for i, (ba, bb) in enumerate(bits):
            for j, (bc, bd) in enumerate(bits):
                coef = (
                    K[ba, :, fs[0], :]
                    * K[bb, :, fs[1], :]
                    * K[bc, :, fs[2], :]
                    * K[bd, :, fs[3], :]
                )  # [R, U]
                C[r0 + i * 4 + j, c0 : c0 + RU] = coef.reshape(RU)
    ident = np.eye(128, dtype=np.float32)
    ones = np.ones((128, CG), np.float32)
    consts = np.concatenate([C, ident, ones], axis=1)
    return consts


_NC_CACHE = {}


def kernel(X: np.ndarray, kernel: np.ndarray) -> np.ndarray:
    if "nc" not in _NC_CACHE:
        _NC_CACHE["nc"] = build_nc()
    nc = _NC_CACHE["nc"]
    consts = _pack_weights(kernel)
    X = np.ascontiguousarray(X, dtype=np.float32)
    # [core, m2, half, chunk, partition, F] -> [core, m2, partition, half*chunk, F]
    Xd = (
        X.reshape(N_CORES, N_M2, 2 * CHUNK, TILE_B, F)
        .transpose(0, 1, 3, 2, 4)
        .copy()
    )
    in_maps = []
    for c in range(N_CORES):
        in_maps.append({"X": Xd[c], "consts": consts})
    res = run_bass_kernel_spmd(nc, in_maps, core_ids=list(range(N_CORES)))

    # host epilogue: product of the four pair-products, sum over CP rank,
    # then apply the normalization S[b] = 1/sqrt(prod_f(1+X^2))
    outs = []
    for c in range(N_CORES):
        o = res.results[c]["out"]  # [N_MACRO, RU, 4, MACRO_B] bf16
        o = np.asarray(o, dtype=np.float32)
        o = o[:, :, 0] * o[:, :, 1] * o[:, :, 2] * o[:, :, 3]
        o = o.reshape(N_MACRO, R, U, MACRO_B).sum(axis=1)  # [N_MACRO, U, 512]
        outs.append(o.transpose(0, 2, 1).reshape(B_CORE, U))
    raw = np.concatenate(outs, axis=0)  # [B, U], rows in device order
    S = 1.0 / np.sqrt(np.prod(1.0 + X.astype(np.float64) ** 2, axis=1))
    return (raw * S[:, None].astype(np.float32)).astype(np.float32)


if __name__ == "__main__":
    rng = np.random.default_rng(0)
    X = rng.standard_normal((B_FULL, F), dtype=np.float32)
    K = (rng.standard_normal((2, R, F, U)) * 0.24).astype(np.float32)
    y = kernel(X, K)
    print(y.shape, y.dtype, np.abs(y).max())
